# revision 26
# baseline (speedup 1.0000x reference)
"""Causal multi-head attention with RoPE, tensor-parallel over heads on 8
Trainium2 NeuronCores.

Problem: B=2, N=2048, C=1024, H=16, D=64.
  q = (x @ Wq) * D^-0.5 ; k = x @ Wk ; v = x @ Wv
  RoPE(q, k); causal softmax(q k^T) v ; out = attn @ Wo

Distribution (tensor-parallel over heads + position-parallel out proj):
  - Core c owns heads 2c, 2c+1 (128 channels of C).
  - Stage 1 (per core): project q,k head-dim-major (transposed) from a
    transposed copy of x; RoPE via a rotation-permutation matmul (P2) plus
    elementwise cos/sin combine; v projected head-dim-major then transposed
    position-major on the PE (identity matmul).
  - Attention per (batch, head): scoresT = k^T-block @ q (both operands
    head-dim-major), unmasked exp (scores are O(1), no overflow), causal
    mask as a 0/1 multiply on the single diagonal 128x128 subtile, PV as
    vaug^T @ exp with a ones column appended to v so the softmax denominator
    falls out of the same matmul; normalize by broadcast divide.
  - AllToAll redistributes attnT so each core holds all 1024 channels for
    its 512 positions; stage 2 computes out_rows = attn[rows] @ Wo.
  - Host concatenates row slices.

All matmul operands bf16 (f32 PSUM accumulation). Measured scale-relative
absmax error vs the f32 reference: ~4e-3 (gate 2e-2).
"""

import numpy as np
import ml_dtypes

B = 2
N_FULL = 2048
C = 1024
H = 16
D = 64
N_CORES = 8
HPC = H // N_CORES      # heads per core = 2
CPC = HPC * D           # channels per core = 128
KC = C // 128           # contraction chunks = 8
IB = 512                # i-block (query block) width
SCALE = D ** -0.5

bf16 = ml_dtypes.bfloat16


def build_nc(n=N_FULL):
    """Build the SPMD Bass program for sequence length n (n=2048 real)."""
    import concourse.bass as bass
    import concourse.mybir as mybir
    import concourse.tile as tile
    from concourse import bacc
    from concourse.masks import make_identity

    dt = mybir.dt
    bn = B * n                    # total positions
    ppc = bn // N_CORES           # positions per core (out rows)
    ib = min(IB, n)
    assert n % ib == 0 and ppc % 128 == 0 and ib % 128 == 0

    nc = bacc.Bacc("TRN2", target_bir_lowering=False, debug=False,
                   num_devices=N_CORES)

    xT = nc.dram_tensor("xT", [C, bn], dt.bfloat16, kind="ExternalInput").ap()
    # host-packed: [128, KC*CPC] with kc-major free dim (row p, col kc*CPC+m
    # holds W[kc*128+p, m])
    wq = nc.dram_tensor("wq", [128, KC * CPC], dt.bfloat16, kind="ExternalInput").ap()
    wk = nc.dram_tensor("wk", [128, KC * CPC], dt.bfloat16, kind="ExternalInput").ap()
    wv = nc.dram_tensor("wv", [128, KC * CPC], dt.bfloat16, kind="ExternalInput").ap()
    wo = nc.dram_tensor("wo", [128, KC * C], dt.bfloat16, kind="ExternalInput").ap()
    cosd = nc.dram_tensor("cosd", [CPC, bn], dt.bfloat16, kind="ExternalInput").ap()
    sind = nc.dram_tensor("sind", [CPC, bn], dt.bfloat16, kind="ExternalInput").ap()
    p2t = nc.dram_tensor("p2t", [CPC, CPC], dt.bfloat16, kind="ExternalInput").ap()
    cmask = nc.dram_tensor("cmask", [128, 128], dt.bfloat16,
                           kind="ExternalInput").ap()
    out = nc.dram_tensor("out", [ppc, C], dt.float32, kind="ExternalOutput").ap()

    nchunk = bn // ib             # stage-1 position chunks

    with tile.TileContext(nc) as tc:
        with (
            tc.tile_pool(name="consts", bufs=1) as consts,
            tc.tile_pool(name="vpool", bufs=1) as vpool,
            tc.tile_pool(name="work", bufs=4) as work,
            tc.tile_pool(name="expp", bufs=12) as expp,
            tc.tile_pool(name="psum", bufs=1, space="PSUM") as psum,
            tc.tile_pool(name="dram", bufs=1, space="DRAM") as dram,
        ):
            # ---- constant loads -------------------------------------------
            # big loads in ~0.5-1 MB pieces: large enough to be
            # bandwidth-bound (DMA issue costs ~650 ns each), small enough
            # that the first projection chunk's inputs land early
            w_sb = {}
            for nm, srcap in (("wq", wq), ("wk", wk), ("wv", wv)):
                t = consts.tile([128, KC * CPC], dt.bfloat16, name=f"{nm}s",
                                tag=f"{nm}s")
                nc.sync.dma_start(out=t, in_=srcap)
                w_sb[nm] = t
            p2_sb = consts.tile([128, CPC], dt.bfloat16, name="p2_sb", tag="p2_sb")
            nc.sync.dma_start(out=p2_sb, in_=p2t)
            tri_sb = consts.tile([128, 128], dt.bfloat16, name="tri_sb",
                                 tag="tri_sb")
            nc.sync.dma_start(out=tri_sb, in_=cmask)
            ident = consts.tile([128, 128], dt.bfloat16, name="ident", tag="ident")
            make_identity(nc, ident)
            xt_sb = [consts.tile([128, bn], dt.bfloat16, name=f"xt{kc}",
                                 tag=f"xt{kc}") for kc in range(KC)]
            cos_sb = consts.tile([128, bn], dt.bfloat16, name="cos_sb", tag="cos_sb")
            sin_sb = consts.tile([128, bn], dt.bfloat16, name="sin_sb", tag="sin_sb")
            nhalf = 2
            for hf in range(nhalf):
                hs_ = slice(hf * bn // nhalf, (hf + 1) * bn // nhalf)
                for kc in range(KC):
                    nc.sync.dma_start(out=xt_sb[kc][:, hs_],
                                      in_=xT[kc * 128:(kc + 1) * 128, hs_])
                nc.sync.dma_start(out=cos_sb[:, hs_], in_=cosd[:, hs_])
                nc.sync.dma_start(out=sin_sb[:, hs_], in_=sind[:, hs_])
            wo_sb = consts.tile([128, KC * C], dt.bfloat16, name="wo_sb",
                                tag="wo_sb")
            for hf in range(2):
                nc.sync.dma_start(
                    out=wo_sb[:, hf * KC * C // 2:(hf + 1) * KC * C // 2],
                    in_=wo[:, hf * KC * C // 2:(hf + 1) * KC * C // 2])

            qr_sb = consts.tile([128, bn], dt.bfloat16, name="qr_sb", tag="qr_sb")
            kr_sb = consts.tile([128, bn], dt.bfloat16, name="kr_sb", tag="kr_sb")

            a2a_in = [dram.tile([N_CORES, D, ppc], dt.bfloat16,
                                 name=f"a2a_in{h}", tag=f"a2a_in{h}")
                      for h in range(HPC)]
            a2a_out = [dram.tile([N_CORES, D, ppc], dt.bfloat16,
                                 name=f"a2a_out{h}", tag=f"a2a_out{h}")
                       for h in range(HPC)]

            # ---- stage 1: q/k/v projections + RoPE, interleaved per chunk
            # (v early so the first attention units can start) --------------
            vaug = []
            for ch in range(nchunk):
                cs = slice(ch * ib, (ch + 1) * ib)
                v_ps = psum.tile([128, ib], dt.float32, name=f"v_ps{ch}",
                                 tag="ps", bufs=8)
                for kc in range(KC):
                    nc.tensor.matmul(
                        v_ps, w_sb["wv"][:, kc * CPC:(kc + 1) * CPC],
                        xt_sb[kc][:, cs],
                        start=(kc == 0), stop=(kc == KC - 1))
                vt_bf = work.tile([128, ib], dt.bfloat16, name="vt_bf",
                                  tag="vt_bf")
                nc.scalar.copy(out=vt_bf, in_=v_ps)
                for sub in range(ib // 128):
                    jc = ch * (ib // 128) + sub
                    va = vpool.tile([128, 2 * (D + 1)], dt.bfloat16,
                                    name=f"va{jc}", tag=f"va{jc}")
                    nc.vector.memset(va, 1.0)
                    tp_ps = psum.tile([128, 128], dt.bfloat16,
                                      name=f"tp{jc}", tag="ps", bufs=8)
                    nc.tensor.transpose(
                        tp_ps, vt_bf[:, sub * 128:(sub + 1) * 128], ident)
                    for h in range(HPC):
                        nc.vector.tensor_copy(
                            out=va[:, h * (D + 1):h * (D + 1) + D],
                            in_=tp_ps[:, h * D:(h + 1) * D])
                    vaug.append(va)
                for nm, dst in (("wq", qr_sb), ("wk", kr_sb)):
                    p_ps = psum.tile([128, ib], dt.float32, name=f"p_{nm}{ch}",
                                     tag="ps", bufs=8)
                    for kc in range(KC):
                        nc.tensor.matmul(
                            p_ps, w_sb[nm][:, kc * CPC:(kc + 1) * CPC],
                            xt_sb[kc][:, cs],
                            start=(kc == 0), stop=(kc == KC - 1))
                    q_bf = work.tile([128, ib], dt.bfloat16, name="q_bf",
                                     tag="q_bf")
                    nc.scalar.copy(out=q_bf, in_=p_ps)
                    rot_ps = psum.tile([128, ib], dt.float32,
                                       name=f"r_{nm}{ch}", tag="ps", bufs=8)
                    nc.tensor.matmul(rot_ps, p2_sb, q_bf, start=True, stop=True)
                    t1 = work.tile([128, ib], dt.float32, name="t1", tag="t1")
                    nc.vector.tensor_mul(t1, q_bf, cos_sb[:, cs])
                    t2 = work.tile([128, ib], dt.float32, name="t2", tag="t2")
                    nc.vector.tensor_mul(t2, rot_ps, sin_sb[:, cs])
                    nc.gpsimd.tensor_add(dst[:, cs], t1, t2)

            # ---- attention (h-major so each head's A2A can start while the
            # next head's attention still computes) -------------------------
            for h in range(HPC):
              for b in range(B):
                for ibk in range(n // ib):
                    i0 = ibk * ib
                    n_j = (i0 + ib) // 128
                    hs = slice(h * D, (h + 1) * D)
                    pv = psum.tile([D + 1, ib], dt.float32,
                                   name=f"pv{b}_{ibk}_{h}", tag="ps", bufs=8)
                    for j in range(n_j):
                        t = j - (n_j - 4)
                        c0 = max(0, t) * 128   # first valid column in i-block
                        s_ps = psum.tile([128, ib], dt.float32,
                                         name=f"s{b}_{ibk}_{j}_{h}",
                                         tag="ps", bufs=8)
                        nc.tensor.matmul(
                            s_ps[:, c0:ib],
                            kr_sb[hs, b * n + j * 128:b * n + (j + 1) * 128],
                            qr_sb[hs, b * n + i0 + c0:b * n + i0 + ib],
                            start=True, stop=True)
                        e_t = expp.tile([128, ib], dt.bfloat16, name="e_t",
                                        tag="e_t")
                        nc.scalar.activation(
                            out=e_t[:, c0:ib], in_=s_ps[:, c0:ib],
                            func=mybir.ActivationFunctionType.Exp)
                        if t >= 0:
                            nc.vector.tensor_mul(
                                e_t[:, c0:c0 + 128], e_t[:, c0:c0 + 128],
                                tri_sb)
                        nc.tensor.matmul(
                            pv[:, c0:ib],
                            vaug[b * (n // 128) + j][:, h * (D + 1):(h + 1) * (D + 1)],
                            e_t[:, c0:ib],
                            start=(j == 0), stop=(j == n_j - 1))
                    den = work.tile([1, ib], dt.float32, name="den", tag="den")
                    nc.vector.tensor_copy(out=den[0:1, :], in_=pv[D:D + 1, :])
                    recip = work.tile([1, ib], dt.float32, name="recip",
                                      tag="recip")
                    nc.vector.reciprocal_approx_fast(
                        out=recip[0:1, :], in_=den[0:1, :])
                    bcr = work.tile([D, ib], dt.float32, name="bcr", tag="bcr")
                    nc.gpsimd.partition_broadcast(bcr[0:D, :], recip[0:1, :])
                    at_t = work.tile([D, ib], dt.bfloat16, name="at_t",
                                     tag="at_t")
                    nc.vector.tensor_mul(at_t, pv[0:D, :], bcr)
                    g0 = b * n + i0
                    for s in range(max(1, ib // ppc)):
                        w = min(ppc, ib)
                        nc.sync.dma_start(
                            out=a2a_in[h][(g0 + s * w) // ppc, 0:D,
                                          (g0 + s * w) % ppc:(g0 + s * w) % ppc + w],
                            in_=at_t[:, s * w:s * w + w])
              # per-head all-to-all: head h's channels redistributed to the
              # position-owning cores; h=0's collective overlaps h=1 compute
              nc.gpsimd.collective_compute(
                  "AllToAll", mybir.AluOpType.bypass,
                  replica_groups=[list(range(N_CORES))],
                  ins=[a2a_in[h].opt()], outs=[a2a_out[h].opt()])

            # ---- stage 2: position-sliced output projection ---------------
            # K (channel) dim split by head: the h0 half only needs A2A#1,
            # so it runs inside the A2A#2 wait window; h1 half completes the
            # PSUM accumulation after A2A#2 lands.
            # ag tiles pack TWO cores' same-head halves -> K=128 matmuls whose
            # h0 chunks depend only on A2A#1 (run in the A2A#2 wait window)
            ag_sb = {}
            for h in range(HPC):
                eng = nc.sync if h == 0 else nc.scalar
                for cpair in range(N_CORES // 2):
                    t = consts.tile([128, ppc], dt.bfloat16,
                                    name=f"ag{h}_{cpair}", tag=f"ag{h}_{cpair}")
                    eng.dma_start(out=t[0:D, :], in_=a2a_out[h][2 * cpair])
                    eng.dma_start(out=t[D:2 * D, :],
                                  in_=a2a_out[h][2 * cpair + 1])
                    ag_sb[(h, cpair)] = t
            o_pss = {}
            for h in range(HPC):
                for m in range(ppc // 128):
                    for nh in range(2):
                        if h == 0:
                            o_pss[(m, nh)] = psum.tile(
                                [128, 512], dt.float32, name=f"o{m}_{nh}",
                                tag="ps", bufs=8)
                        o_ps = o_pss[(m, nh)]
                        for cpair in range(N_CORES // 2):
                            ck = h * 4 + cpair
                            nc.tensor.matmul(
                                o_ps,
                                ag_sb[(h, cpair)][:, m * 128:(m + 1) * 128],
                                wo_sb[:, ck * C + nh * 512:ck * C + (nh + 1) * 512],
                                start=(h == 0 and cpair == 0),
                                stop=(h == 1 and cpair == N_CORES // 2 - 1))
            for m in range(ppc // 128):
                o_sb = work.tile([128, C], dt.float32, name="o_sb",
                                 tag="o_sb", bufs=2)
                for nh in range(2):
                    nc.vector.tensor_copy(
                        out=o_sb[:, nh * 512:(nh + 1) * 512],
                        in_=o_pss[(m, nh)])
                nc.sync.dma_start(out=out[m * 128:(m + 1) * 128, :], in_=o_sb)

    nc.compile()
    return nc


def host_inputs(hidden_states, cos, sin, Wq, Wk, Wv, Wo, n=N_FULL):
    """Build the 8 per-core input maps from full inputs."""
    bn = B * n
    x = np.ascontiguousarray(hidden_states.reshape(bn, C))
    xT = x.T.astype(bf16)

    cosf = cos.reshape(bn, D).T
    sinf = sin.reshape(bn, D).T
    cosd = np.ascontiguousarray(np.concatenate([cosf, cosf], axis=0)).astype(bf16)
    sind = np.ascontiguousarray(np.concatenate([sinf, sinf], axis=0)).astype(bf16)

    # rotation permutation: (P2 q)[p] = rotate_half(q)[p], block-diag per head
    p2 = np.zeros((CPC, CPC), np.float32)
    for h in range(HPC):
        s = h * D
        for pl in range(D // 2):
            p2[s + pl, s + pl + D // 2] = -1.0
            p2[s + pl + D // 2, s + pl] = 1.0
    p2t = np.ascontiguousarray(p2.T).astype(bf16)

    pj = np.arange(128)[:, None]
    fi = np.arange(128)[None, :]
    cmask = (pj <= fi).astype(bf16)                       # (128, 128)

    def pack(w):
        # (C, M) -> (128, KC*M): row p, col kc*M+m = w[kc*128+p, m]
        m = w.shape[1]
        return np.ascontiguousarray(
            w.reshape(KC, 128, m).transpose(1, 0, 2).reshape(128, KC * m)
        ).astype(bf16)

    # stage-2 packing: chunk ck pairs cores (2c, 2c+1)'s head-h channel
    # halves: row p<64 -> Wo[(2c)*128 + h*64 + p], p>=64 -> Wo[(2c+1)*128
    # + h*64 + (p-64)]
    wo_rows = np.empty((8, 128), np.int64)
    for ck in range(8):
        h, cpair = ck // 4, ck % 4
        wo_rows[ck, :64] = (2 * cpair) * 128 + h * 64 + np.arange(64)
        wo_rows[ck, 64:] = (2 * cpair + 1) * 128 + h * 64 + np.arange(64)
    wo_s = np.ascontiguousarray(
        Wo[wo_rows.reshape(-1)].reshape(8, 128, C).transpose(1, 0, 2)
        .reshape(128, 8 * C)).astype(bf16)

    in_maps = []
    for c in range(N_CORES):
        cols = slice(c * CPC, (c + 1) * CPC)
        in_maps.append({
            "xT": xT,
            "wq": pack(Wq[:, cols] * SCALE),
            "wk": pack(Wk[:, cols]),
            "wv": pack(Wv[:, cols]),
            "wo": wo_s,
            "cosd": cosd,
            "sind": sind,
            "p2t": p2t,
            "cmask": cmask,
        })
    return in_maps


_NC_CACHE = {}


def kernel(hidden_states, cos, sin, Wq, Wk, Wv, Wo):
    from concourse.bass_utils import run_bass_kernel_spmd

    if N_FULL not in _NC_CACHE:
        _NC_CACHE[N_FULL] = build_nc(N_FULL)
    nc = _NC_CACHE[N_FULL]

    in_maps = host_inputs(hidden_states, cos, sin, Wq, Wk, Wv, Wo)
    res = run_bass_kernel_spmd(nc, in_maps, core_ids=list(range(N_CORES)))
    out = np.concatenate([r["out"] for r in res.results], axis=0)
    return out.reshape(B, N_FULL, C).astype(np.float32)


# revision 27
# speedup vs baseline: 1.0135x; 1.0135x over previous
"""Causal multi-head attention with RoPE, tensor-parallel over heads on 8
Trainium2 NeuronCores.

Problem: B=2, N=2048, C=1024, H=16, D=64.
  q = (x @ Wq) * D^-0.5 ; k = x @ Wk ; v = x @ Wv
  RoPE(q, k); causal softmax(q k^T) v ; out = attn @ Wo

Distribution (tensor-parallel over heads + position-parallel out proj):
  - Core c owns heads 2c, 2c+1 (128 channels of C).
  - Stage 1 (per core): project q,k head-dim-major (transposed) from a
    transposed copy of x; RoPE via a rotation-permutation matmul (P2) plus
    elementwise cos/sin combine; v projected head-dim-major then transposed
    position-major on the PE (identity matmul).
  - Attention per (batch, head): scoresT = k^T-block @ q (both operands
    head-dim-major), unmasked exp (scores are O(1), no overflow), causal
    mask as a 0/1 multiply on the single diagonal 128x128 subtile, PV as
    vaug^T @ exp with a ones column appended to v so the softmax denominator
    falls out of the same matmul; normalize by broadcast divide.
  - AllToAll redistributes attnT so each core holds all 1024 channels for
    its 512 positions; stage 2 computes out_rows = attn[rows] @ Wo.
  - Host concatenates row slices.

All matmul operands bf16 (f32 PSUM accumulation). Measured scale-relative
absmax error vs the f32 reference: ~4e-3 (gate 2e-2).
"""

import numpy as np
import ml_dtypes

B = 2
N_FULL = 2048
C = 1024
H = 16
D = 64
N_CORES = 8
HPC = H // N_CORES      # heads per core = 2
CPC = HPC * D           # channels per core = 128
KC = C // 128           # contraction chunks = 8
IB = 512                # i-block (query block) width
SCALE = D ** -0.5

bf16 = ml_dtypes.bfloat16


def build_nc(n=N_FULL):
    """Build the SPMD Bass program for sequence length n (n=2048 real)."""
    import concourse.bass as bass
    import concourse.mybir as mybir
    import concourse.tile as tile
    from concourse import bacc
    from concourse.masks import make_identity

    dt = mybir.dt
    bn = B * n                    # total positions
    ppc = bn // N_CORES           # positions per core (out rows)
    ib = min(IB, n)
    assert n % ib == 0 and ppc % 128 == 0 and ib % 128 == 0

    nc = bacc.Bacc("TRN2", target_bir_lowering=False, debug=False,
                   num_devices=N_CORES)

    xT = nc.dram_tensor("xT", [C, bn], dt.bfloat16, kind="ExternalInput").ap()
    # host-packed: [128, KC*CPC] with kc-major free dim (row p, col kc*CPC+m
    # holds W[kc*128+p, m])
    wq = nc.dram_tensor("wq", [128, KC * CPC], dt.bfloat16, kind="ExternalInput").ap()
    wk = nc.dram_tensor("wk", [128, KC * CPC], dt.bfloat16, kind="ExternalInput").ap()
    wv = nc.dram_tensor("wv", [128, KC * CPC], dt.bfloat16, kind="ExternalInput").ap()
    wo = nc.dram_tensor("wo", [128, KC * C], dt.bfloat16, kind="ExternalInput").ap()
    cosd = nc.dram_tensor("cosd", [CPC, bn], dt.bfloat16, kind="ExternalInput").ap()
    sind = nc.dram_tensor("sind", [CPC, bn], dt.bfloat16, kind="ExternalInput").ap()
    p2t = nc.dram_tensor("p2t", [CPC, CPC], dt.bfloat16, kind="ExternalInput").ap()
    cmask = nc.dram_tensor("cmask", [128, 128], dt.bfloat16,
                           kind="ExternalInput").ap()
    out = nc.dram_tensor("out", [ppc, C], dt.float32, kind="ExternalOutput").ap()

    nchunk = bn // ib             # stage-1 position chunks

    with tile.TileContext(nc) as tc:
        with (
            tc.tile_pool(name="consts", bufs=1) as consts,
            tc.tile_pool(name="vpool", bufs=1) as vpool,
            tc.tile_pool(name="work", bufs=4) as work,
            tc.tile_pool(name="expp", bufs=12) as expp,
            tc.tile_pool(name="psum", bufs=1, space="PSUM") as psum,
            tc.tile_pool(name="dram", bufs=1, space="DRAM") as dram,
        ):
            # ---- constant loads -------------------------------------------
            # big loads in ~0.5-1 MB pieces: large enough to be
            # bandwidth-bound (DMA issue costs ~650 ns each), small enough
            # that the first projection chunk's inputs land early
            w_sb = {}
            for nm, srcap in (("wq", wq), ("wk", wk), ("wv", wv)):
                t = consts.tile([128, KC * CPC], dt.bfloat16, name=f"{nm}s",
                                tag=f"{nm}s")
                nc.sync.dma_start(out=t, in_=srcap)
                w_sb[nm] = t
            p2_sb = consts.tile([128, CPC], dt.bfloat16, name="p2_sb", tag="p2_sb")
            nc.sync.dma_start(out=p2_sb, in_=p2t)
            tri_sb = consts.tile([128, 128], dt.bfloat16, name="tri_sb",
                                 tag="tri_sb")
            nc.sync.dma_start(out=tri_sb, in_=cmask)
            ident = consts.tile([128, 128], dt.bfloat16, name="ident", tag="ident")
            make_identity(nc, ident)
            xt_sb = [consts.tile([128, bn], dt.bfloat16, name=f"xt{kc}",
                                 tag=f"xt{kc}") for kc in range(KC)]
            cos_sb = consts.tile([128, bn], dt.bfloat16, name="cos_sb", tag="cos_sb")
            sin_sb = consts.tile([128, bn], dt.bfloat16, name="sin_sb", tag="sin_sb")
            for kc in range(KC):
                nc.sync.dma_start(out=xt_sb[kc][:, 0:ib],
                                  in_=xT[kc * 128:(kc + 1) * 128, 0:ib])
            nc.sync.dma_start(out=cos_sb[:, 0:ib], in_=cosd[:, 0:ib])
            nc.sync.dma_start(out=sin_sb[:, 0:ib], in_=sind[:, 0:ib])
            nhalf = 2
            for hf in range(nhalf):
                lo = ib + hf * (bn - ib) // nhalf
                hi = ib + (hf + 1) * (bn - ib) // nhalf
                for kc in range(KC):
                    nc.sync.dma_start(out=xt_sb[kc][:, lo:hi],
                                      in_=xT[kc * 128:(kc + 1) * 128, lo:hi])
                nc.sync.dma_start(out=cos_sb[:, lo:hi], in_=cosd[:, lo:hi])
                nc.sync.dma_start(out=sin_sb[:, lo:hi], in_=sind[:, lo:hi])
            wo_sb = consts.tile([128, KC * C], dt.bfloat16, name="wo_sb",
                                tag="wo_sb")
            for hf in range(2):
                nc.sync.dma_start(
                    out=wo_sb[:, hf * KC * C // 2:(hf + 1) * KC * C // 2],
                    in_=wo[:, hf * KC * C // 2:(hf + 1) * KC * C // 2])

            qr_sb = consts.tile([128, bn], dt.bfloat16, name="qr_sb", tag="qr_sb")
            kr_sb = consts.tile([128, bn], dt.bfloat16, name="kr_sb", tag="kr_sb")

            a2a_in = [dram.tile([N_CORES, D, ppc], dt.bfloat16,
                                 name=f"a2a_in{h}", tag=f"a2a_in{h}")
                      for h in range(HPC)]
            a2a_out = [dram.tile([N_CORES, D, ppc], dt.bfloat16,
                                 name=f"a2a_out{h}", tag=f"a2a_out{h}")
                       for h in range(HPC)]

            # ---- stage 1: q/k/v projections + RoPE, interleaved per chunk
            # (v early so the first attention units can start) --------------
            vaug = []
            for ch in range(nchunk):
                cs = slice(ch * ib, (ch + 1) * ib)
                v_ps = psum.tile([128, ib], dt.float32, name=f"v_ps{ch}",
                                 tag="ps", bufs=8)
                for kc in range(KC):
                    nc.tensor.matmul(
                        v_ps, w_sb["wv"][:, kc * CPC:(kc + 1) * CPC],
                        xt_sb[kc][:, cs],
                        start=(kc == 0), stop=(kc == KC - 1))
                vt_bf = work.tile([128, ib], dt.bfloat16, name="vt_bf",
                                  tag="vt_bf")
                nc.scalar.copy(out=vt_bf, in_=v_ps)
                tp_ps = psum.tile([128, ib], dt.bfloat16,
                                  name=f"tp{ch}", tag="ps", bufs=8)
                for sub in range(ib // 128):
                    jc = ch * (ib // 128) + sub
                    va = vpool.tile([128, 2 * (D + 1)], dt.bfloat16,
                                    name=f"va{jc}", tag=f"va{jc}")
                    nc.vector.memset(va, 1.0)
                    nc.tensor.transpose(
                        tp_ps[:, sub * 128:(sub + 1) * 128],
                        vt_bf[:, sub * 128:(sub + 1) * 128], ident)
                    for h in range(HPC):
                        nc.vector.tensor_copy(
                            out=va[:, h * (D + 1):h * (D + 1) + D],
                            in_=tp_ps[:, sub * 128 + h * D:sub * 128 + (h + 1) * D])
                    vaug.append(va)
                for nm, dst in (("wq", qr_sb), ("wk", kr_sb)):
                    p_ps = psum.tile([128, ib], dt.float32, name=f"p_{nm}{ch}",
                                     tag="ps", bufs=8)
                    for kc in range(KC):
                        nc.tensor.matmul(
                            p_ps, w_sb[nm][:, kc * CPC:(kc + 1) * CPC],
                            xt_sb[kc][:, cs],
                            start=(kc == 0), stop=(kc == KC - 1))
                    q_bf = work.tile([128, ib], dt.bfloat16, name="q_bf",
                                     tag="q_bf")
                    nc.scalar.copy(out=q_bf, in_=p_ps)
                    rot_ps = psum.tile([128, ib], dt.float32,
                                       name=f"r_{nm}{ch}", tag="ps", bufs=8)
                    nc.tensor.matmul(rot_ps, p2_sb, q_bf, start=True, stop=True)
                    t1 = work.tile([128, ib], dt.float32, name="t1", tag="t1")
                    nc.vector.tensor_mul(t1, q_bf, cos_sb[:, cs])
                    t2 = work.tile([128, ib], dt.float32, name="t2", tag="t2")
                    nc.vector.tensor_mul(t2, rot_ps, sin_sb[:, cs])
                    nc.gpsimd.tensor_add(dst[:, cs], t1, t2)

            # ---- attention (h-major so each head's A2A can start while the
            # next head's attention still computes) -------------------------
            for h in range(HPC):
              for b in range(B):
                for ibk in range(n // ib):
                    i0 = ibk * ib
                    n_j = (i0 + ib) // 128
                    hs = slice(h * D, (h + 1) * D)
                    pv = psum.tile([D + 1, ib], dt.float32,
                                   name=f"pv{b}_{ibk}_{h}", tag="ps", bufs=8)
                    for j in range(n_j):
                        t = j - (n_j - 4)
                        c0 = max(0, t) * 128   # first valid column in i-block
                        s_ps = psum.tile([128, ib], dt.float32,
                                         name=f"s{b}_{ibk}_{j}_{h}",
                                         tag="ps", bufs=8)
                        nc.tensor.matmul(
                            s_ps[:, c0:ib],
                            kr_sb[hs, b * n + j * 128:b * n + (j + 1) * 128],
                            qr_sb[hs, b * n + i0 + c0:b * n + i0 + ib],
                            start=True, stop=True)
                        e_t = expp.tile([128, ib], dt.bfloat16, name="e_t",
                                        tag="e_t")
                        nc.scalar.activation(
                            out=e_t[:, c0:ib], in_=s_ps[:, c0:ib],
                            func=mybir.ActivationFunctionType.Exp)
                        if t >= 0:
                            nc.vector.tensor_mul(
                                e_t[:, c0:c0 + 128], e_t[:, c0:c0 + 128],
                                tri_sb)
                        nc.tensor.matmul(
                            pv[:, c0:ib],
                            vaug[b * (n // 128) + j][:, h * (D + 1):(h + 1) * (D + 1)],
                            e_t[:, c0:ib],
                            start=(j == 0), stop=(j == n_j - 1))
                    den = work.tile([1, ib], dt.float32, name="den", tag="den")
                    nc.vector.tensor_copy(out=den[0:1, :], in_=pv[D:D + 1, :])
                    recip = work.tile([1, ib], dt.float32, name="recip",
                                      tag="recip")
                    nc.vector.reciprocal_approx_fast(
                        out=recip[0:1, :], in_=den[0:1, :])
                    bcr = work.tile([D, ib], dt.float32, name="bcr", tag="bcr")
                    nc.gpsimd.partition_broadcast(bcr[0:D, :], recip[0:1, :])
                    at_t = work.tile([D, ib], dt.bfloat16, name="at_t",
                                     tag="at_t")
                    nc.vector.tensor_mul(at_t, pv[0:D, :], bcr)
                    g0 = b * n + i0
                    for s in range(max(1, ib // ppc)):
                        w = min(ppc, ib)
                        nc.sync.dma_start(
                            out=a2a_in[h][(g0 + s * w) // ppc, 0:D,
                                          (g0 + s * w) % ppc:(g0 + s * w) % ppc + w],
                            in_=at_t[:, s * w:s * w + w])
              # per-head all-to-all: head h's channels redistributed to the
              # position-owning cores; h=0's collective overlaps h=1 compute
              nc.gpsimd.collective_compute(
                  "AllToAll", mybir.AluOpType.bypass,
                  replica_groups=[list(range(N_CORES))],
                  ins=[a2a_in[h].opt()], outs=[a2a_out[h].opt()])

            # ---- stage 2: position-sliced output projection ---------------
            # K (channel) dim split by head: the h0 half only needs A2A#1,
            # so it runs inside the A2A#2 wait window; h1 half completes the
            # PSUM accumulation after A2A#2 lands.
            # ag tiles pack TWO cores' same-head halves -> K=128 matmuls whose
            # h0 chunks depend only on A2A#1 (run in the A2A#2 wait window)
            ag_sb = {}
            for h in range(HPC):
                eng = nc.sync if h == 0 else nc.scalar
                for cpair in range(N_CORES // 2):
                    t = consts.tile([128, ppc], dt.bfloat16,
                                    name=f"ag{h}_{cpair}", tag=f"ag{h}_{cpair}")
                    eng.dma_start(out=t[0:D, :], in_=a2a_out[h][2 * cpair])
                    eng.dma_start(out=t[D:2 * D, :],
                                  in_=a2a_out[h][2 * cpair + 1])
                    ag_sb[(h, cpair)] = t
            o_pss = {}
            for h in range(HPC):
                for m in range(ppc // 128):
                    for nh in range(2):
                        if h == 0:
                            o_pss[(m, nh)] = psum.tile(
                                [128, 512], dt.float32, name=f"o{m}_{nh}",
                                tag="ps", bufs=8)
                        o_ps = o_pss[(m, nh)]
                        for cpair in range(N_CORES // 2):
                            ck = h * 4 + cpair
                            nc.tensor.matmul(
                                o_ps,
                                ag_sb[(h, cpair)][:, m * 128:(m + 1) * 128],
                                wo_sb[:, ck * C + nh * 512:ck * C + (nh + 1) * 512],
                                start=(h == 0 and cpair == 0),
                                stop=(h == 1 and cpair == N_CORES // 2 - 1))
            for m in range(ppc // 128):
                o_sb = work.tile([128, C], dt.float32, name="o_sb",
                                 tag="o_sb", bufs=2)
                for nh in range(2):
                    nc.vector.tensor_copy(
                        out=o_sb[:, nh * 512:(nh + 1) * 512],
                        in_=o_pss[(m, nh)])
                nc.sync.dma_start(out=out[m * 128:(m + 1) * 128, :], in_=o_sb)

    nc.compile()
    return nc


def host_inputs(hidden_states, cos, sin, Wq, Wk, Wv, Wo, n=N_FULL):
    """Build the 8 per-core input maps from full inputs."""
    bn = B * n
    x = np.ascontiguousarray(hidden_states.reshape(bn, C))
    xT = x.T.astype(bf16)

    cosf = cos.reshape(bn, D).T
    sinf = sin.reshape(bn, D).T
    cosd = np.ascontiguousarray(np.concatenate([cosf, cosf], axis=0)).astype(bf16)
    sind = np.ascontiguousarray(np.concatenate([sinf, sinf], axis=0)).astype(bf16)

    # rotation permutation: (P2 q)[p] = rotate_half(q)[p], block-diag per head
    p2 = np.zeros((CPC, CPC), np.float32)
    for h in range(HPC):
        s = h * D
        for pl in range(D // 2):
            p2[s + pl, s + pl + D // 2] = -1.0
            p2[s + pl + D // 2, s + pl] = 1.0
    p2t = np.ascontiguousarray(p2.T).astype(bf16)

    pj = np.arange(128)[:, None]
    fi = np.arange(128)[None, :]
    cmask = (pj <= fi).astype(bf16)                       # (128, 128)

    def pack(w):
        # (C, M) -> (128, KC*M): row p, col kc*M+m = w[kc*128+p, m]
        m = w.shape[1]
        return np.ascontiguousarray(
            w.reshape(KC, 128, m).transpose(1, 0, 2).reshape(128, KC * m)
        ).astype(bf16)

    # stage-2 packing: chunk ck pairs cores (2c, 2c+1)'s head-h channel
    # halves: row p<64 -> Wo[(2c)*128 + h*64 + p], p>=64 -> Wo[(2c+1)*128
    # + h*64 + (p-64)]
    wo_rows = np.empty((8, 128), np.int64)
    for ck in range(8):
        h, cpair = ck // 4, ck % 4
        wo_rows[ck, :64] = (2 * cpair) * 128 + h * 64 + np.arange(64)
        wo_rows[ck, 64:] = (2 * cpair + 1) * 128 + h * 64 + np.arange(64)
    wo_s = np.ascontiguousarray(
        Wo[wo_rows.reshape(-1)].reshape(8, 128, C).transpose(1, 0, 2)
        .reshape(128, 8 * C)).astype(bf16)

    in_maps = []
    for c in range(N_CORES):
        cols = slice(c * CPC, (c + 1) * CPC)
        in_maps.append({
            "xT": xT,
            "wq": pack(Wq[:, cols] * SCALE),
            "wk": pack(Wk[:, cols]),
            "wv": pack(Wv[:, cols]),
            "wo": wo_s,
            "cosd": cosd,
            "sind": sind,
            "p2t": p2t,
            "cmask": cmask,
        })
    return in_maps


_NC_CACHE = {}


def kernel(hidden_states, cos, sin, Wq, Wk, Wv, Wo):
    from concourse.bass_utils import run_bass_kernel_spmd

    if N_FULL not in _NC_CACHE:
        _NC_CACHE[N_FULL] = build_nc(N_FULL)
    nc = _NC_CACHE[N_FULL]

    in_maps = host_inputs(hidden_states, cos, sin, Wq, Wk, Wv, Wo)
    res = run_bass_kernel_spmd(nc, in_maps, core_ids=list(range(N_CORES)))
    out = np.concatenate([r["out"] for r in res.results], axis=0)
    return out.reshape(B, N_FULL, C).astype(np.float32)
